# revision 10
# baseline (speedup 1.0000x reference)
"""Ragged -> padded batch scatter (BatchedSequences) on 8 TRN2 NeuronCores.

Reference semantics: rows of concatenated_sequences [T, F] are scattered into
a zero-padded output [B, max_sl, F] according to per-sequence lengths.

Strategy (pure data movement, memory-bound; per-NC HBM r+w sustains
~665 GB/s, so the floor for 2*25.4 MB per core is ~76.5 us of data phase
plus ~9.5 us framework ramp and ~3 us drain; measured 88.5-91 us per core
on quiet NCs, up to ~107 on NCs with neighbor HBM load. The previous
SBUF-staged indirect-scatter kernel measured 156 us: every byte crossed
the SDMA engines twice, capping at 16 engines x 26 GB/s):
  - Shard 4 sequences per core with the slot assignment
    groups[k] = [k, 15-k, 16+k, 31-k]; every core then owns exactly
    T/8 = 12416 rows (lengths decay linearly, slots pair them off).
  - Slot j's length varies per core, but its minimum over cores is a
    static "base" size (3648/3136/2624/2112 rows = 11520 of 12416 rows).
    Base pieces are copied with 4 big *direct DRAM->DRAM* DMAs whose
    sizes/offsets are identical on all 8 cores (pure SPMD), split over
    the two HWDGE rings (sync + scalar). No SBUF transit: each byte
    crosses the SDMA engines once instead of twice.
  - The ragged remainder (896 rows/core = 14 pieces of 64 rows) is also
    DRAM->DRAM: gpsimd loads each piece's destination row from a tiny
    per-core int32 table (SBUF) into a register and issues a
    dynamic-offset SWDGE copy. These interleave with the big copies on
    the SDMA engines mid-stream.
  - Host pre-arranges each core's x so all source offsets are static:
    [base_0 | base_1 | base_2 | base_3 | tail_0 | tail_1 | tail_2 | tail_3].
  - Padding stays zero because run_bass_kernel_spmd pre-zeroes / donates
    zero-filled ExternalOutput buffers.
"""

from contextlib import ExitStack

import numpy as np

import concourse.bass as bass
import concourse.mybir as mybir
from concourse.bass_utils import run_bass_kernel_spmd

B = 32
F = 512
MAX_SL = 4096
NCORES = 8
SEQ_PER_CORE = B // NCORES
RU = 64                         # remainder piece: 64 rows = 128 KiB

_NC_CACHE: dict[tuple, bass.Bass] = {}


def _build_nc(bases: tuple[int, ...], rem_rows: int) -> bass.Bass:
    """Uniform per-core program (see module docstring)."""
    nc = bass.Bass(monotonic_sem_count=0, enable_partition_id=False)
    n_base = sum(bases)
    n_rows = n_base + rem_rows
    n_rem = rem_rows // RU
    x = nc.declare_dram_parameter("x", [n_rows, F], mybir.dt.float32, isOutput=False)
    dst = nc.declare_dram_parameter("dst", [1, n_rem], mybir.dt.int32, isOutput=False)
    y = nc.declare_dram_parameter(
        "y", [SEQ_PER_CORE * MAX_SL, F], mybir.dt.float32, isOutput=True
    )

    src_off = [0]
    for bj in bases:
        src_off.append(src_off[-1] + bj)

    with ExitStack() as ctx:
        dst_t = ctx.enter_context(nc.sbuf_tensor([1, n_rem], mybir.dt.int32))
        sem_tab = ctx.enter_context(nc.semaphore("sem_tab"))
        sem_big = ctx.enter_context(nc.semaphore("sem_big"))
        sem_rem = ctx.enter_context(nc.semaphore("sem_rem"))
        block = ctx.enter_context(nc.Block())

        LEAD = 128  # small lead-off piece: fast descriptor gen -> engines start sooner

        def big(eng, j, lead=0):
            if lead:
                eng.dma_start(
                    out=y[j * MAX_SL : j * MAX_SL + lead, :],
                    in_=x[src_off[j] : src_off[j] + lead, :],
                ).then_inc(sem_big, 16)
            eng.dma_start(
                out=y[j * MAX_SL + lead : j * MAX_SL + bases[j], :],
                in_=x[src_off[j] + lead : src_off[j] + bases[j], :],
            ).then_inc(sem_big, 16)

        def smalls(eng, ms, r):
            # dynamic-offset DRAM->DRAM remainder copies on the HWDGE ring,
            # placed mid-ring so they drain between the big copies
            for m in ms:
                eng.reg_load(r, dst_t[0:1, m : m + 1])
                v = eng.snap(r, min_val=0, max_val=SEQ_PER_CORE * MAX_SL - RU)
                eng.dma_start(
                    out=y[bass.ds(v, RU), :],
                    in_=x[n_base + m * RU : n_base + (m + 1) * RU, :],
                ).then_inc(sem_rem, 16)

        half = n_rem // 2

        @block.scalar
        def _(scalar):
            # tiny table load first (unblocks the remainder copies); its
            # completion stall is hidden behind the already-issued big copies
            scalar.dma_start(out=dst_t[:, :], in_=dst[:, :]).then_inc(sem_tab, 16)
            big(scalar, 1, lead=LEAD)
            scalar.wait_ge(sem_tab, 16)
            with scalar.register("dst_row_act") as r:
                smalls(scalar, range(half, n_rem), r)
            big(scalar, 3)

        @block.sync
        def _(sync):
            big(sync, 0, lead=LEAD)
            sync.wait_ge(sem_tab, 16)
            with sync.register("dst_row_sp") as r:
                smalls(sync, range(half), r)
            big(sync, 2)
            sync.wait_ge(sem_big, 16 * (len(bases) + 2))
            sync.wait_ge(sem_rem, 16 * n_rem)
    return nc


def _groups():
    return [[k, 15 - k, 16 + k, 31 - k] for k in range(NCORES)]


def _host_fallback(S, L, max_sl):
    out = np.zeros((len(L), max_sl, S.shape[1]), dtype=S.dtype)
    off = 0
    for b, ln in enumerate(L):
        out[b, :ln] = S[off : off + ln]
        off += ln
    return out


def _fast_path_ok(S, L, max_sl):
    if (
        max_sl != MAX_SL
        or len(L) != B
        or S.shape[1] != F
        or int(L.sum()) != S.shape[0]
        or np.any(L % RU)
        or np.any(L < RU)
        or np.any(L > max_sl)
    ):
        return False
    groups = _groups()
    totals = [sum(int(L[s]) for s in g) for g in groups]
    if len(set(totals)) != 1:
        return False
    bases = [min(int(L[g[j]]) for g in groups) for j in range(SEQ_PER_CORE)]
    rem = totals[0] - sum(bases)
    if rem % RU or not (1 <= rem // RU <= 64):
        return False
    return True


def _prepare(S, L):
    offsets = np.zeros(B + 1, dtype=np.int64)
    np.cumsum(L, out=offsets[1:])
    groups = _groups()
    bases = [min(int(L[g[j]]) for g in groups) for j in range(SEQ_PER_CORE)]
    rem_rows = sum(int(L[s]) for s in groups[0]) - sum(bases)
    n_rem = rem_rows // RU

    in_maps = []
    for k in range(NCORES):
        xs = []
        tails = []
        dst_k = np.zeros((1, n_rem), dtype=np.int32)
        p = 0
        for j, s in enumerate(groups[k]):
            ln = int(L[s])
            bj = bases[j]
            xs.append(S[offsets[s] : offsets[s] + bj])
            tails.append(S[offsets[s] + bj : offsets[s] + ln])
            for u in range((ln - bj) // RU):
                dst_k[0, p] = j * MAX_SL + bj + u * RU
                p += 1
        assert p == n_rem
        x_k = np.concatenate(xs + tails, axis=0)
        in_maps.append({"x": x_k, "dst": dst_k})

    key = (tuple(bases), rem_rows)
    if key not in _NC_CACHE:
        _NC_CACHE[key] = _build_nc(*key)
    return _NC_CACHE[key], in_maps, groups


def _assemble(results, groups):
    out = np.empty((B, MAX_SL, F), dtype=np.float32)
    for k in range(NCORES):
        yk = np.asarray(results[k]["y"]).reshape(SEQ_PER_CORE, MAX_SL, F)
        for j, s in enumerate(groups[k]):
            out[s] = yk[j]
    return out


def kernel(concatenated_sequences, sequence_lengths, max_sl):
    S = np.ascontiguousarray(np.asarray(concatenated_sequences, dtype=np.float32))
    L = np.asarray(sequence_lengths).reshape(-1).astype(np.int64)
    max_sl = int(np.asarray(max_sl))

    if not _fast_path_ok(S, L, max_sl):
        return _host_fallback(S, L, max_sl)

    nc, in_maps, groups = _prepare(S, L)
    res = run_bass_kernel_spmd(nc, in_maps, list(range(NCORES))).results
    return _assemble(res, groups)


# revision 12
# speedup vs baseline: 1.1746x; 1.1746x over previous
"""Ragged -> padded batch scatter (BatchedSequences) on 8 TRN2 NeuronCores.

Reference semantics: rows of concatenated_sequences [T, F] are scattered into
a zero-padded output [B, max_sl, F] according to per-sequence lengths.

Strategy (pure data movement, memory-bound; per-NC HBM r+w sustains
~665 GB/s, so the floor for 2*25.4 MB per core is ~76.5 us of data phase
plus ~9.5 us framework ramp and ~3 us drain; measured 88.5-91 us per core
on quiet NCs, up to ~107 on NCs with neighbor HBM load. The previous
SBUF-staged indirect-scatter kernel measured 156 us: every byte crossed
the SDMA engines twice, capping at 16 engines x 26 GB/s):
  - Shard 4 sequences per core with the slot assignment
    groups[k] = [k, 15-k, 16+k, 31-k]; every core then owns exactly
    T/8 = 12416 rows (lengths decay linearly, slots pair them off).
  - Slot j's length varies per core, but its minimum over cores is a
    static "base" size (3648/3136/2624/2112 rows = 11520 of 12416 rows).
    Base pieces are copied with 4 big *direct DRAM->DRAM* DMAs whose
    sizes/offsets are identical on all 8 cores (pure SPMD), split over
    the two HWDGE rings (sync + scalar). No SBUF transit: each byte
    crosses the SDMA engines once instead of twice.
  - The ragged remainder (896 rows/core = 14 pieces of 64 rows) is also
    DRAM->DRAM: gpsimd loads each piece's destination row from a tiny
    per-core int32 table (SBUF) into a register and issues a
    dynamic-offset SWDGE copy. These interleave with the big copies on
    the SDMA engines mid-stream.
  - Host pre-arranges each core's x so all source offsets are static:
    [base_0 | base_1 | base_2 | base_3 | tail_0 | tail_1 | tail_2 | tail_3].
  - Padding stays zero because run_bass_kernel_spmd pre-zeroes / donates
    zero-filled ExternalOutput buffers.
"""

from contextlib import ExitStack

import numpy as np

import concourse.bass as bass
import concourse.mybir as mybir
from concourse.bass_utils import run_bass_kernel_spmd

B = 32
F = 512
MAX_SL = 4096
NCORES = 8
SEQ_PER_CORE = B // NCORES
RU = 64                         # remainder piece: 64 rows = 128 KiB

_NC_CACHE: dict[tuple, bass.Bass] = {}


def _build_nc(bases: tuple[int, ...], rem_rows: int) -> bass.Bass:
    """Uniform per-core program (see module docstring)."""
    nc = bass.Bass()
    n_base = sum(bases)
    n_rows = n_base + rem_rows
    n_rem = rem_rows // RU
    x = nc.declare_dram_parameter("x", [n_rows, F], mybir.dt.float32, isOutput=False)
    dst = nc.declare_dram_parameter("dst", [1, n_rem], mybir.dt.int32, isOutput=False)
    y = nc.declare_dram_parameter(
        "y", [SEQ_PER_CORE * MAX_SL, F], mybir.dt.float32, isOutput=True
    )

    src_off = [0]
    for bj in bases:
        src_off.append(src_off[-1] + bj)

    with ExitStack() as ctx:
        dst_t = ctx.enter_context(nc.sbuf_tensor([1, n_rem], mybir.dt.int32))
        sem_tab = ctx.enter_context(nc.semaphore("sem_tab"))
        sem_big = ctx.enter_context(nc.semaphore("sem_big"))
        sem_rem = ctx.enter_context(nc.semaphore("sem_rem"))
        block = ctx.enter_context(nc.Block())

        LEAD = 128  # small lead-off piece: fast descriptor gen -> engines start sooner

        def big(eng, j, lead=0):
            if lead:
                eng.dma_start(
                    out=y[j * MAX_SL : j * MAX_SL + lead, :],
                    in_=x[src_off[j] : src_off[j] + lead, :],
                ).then_inc(sem_big, 16)
            eng.dma_start(
                out=y[j * MAX_SL + lead : j * MAX_SL + bases[j], :],
                in_=x[src_off[j] + lead : src_off[j] + bases[j], :],
            ).then_inc(sem_big, 16)

        def smalls(eng, ms, r):
            # dynamic-offset DRAM->DRAM remainder copies on the HWDGE ring,
            # placed mid-ring so they drain between the big copies
            for m in ms:
                eng.reg_load(r, dst_t[0:1, m : m + 1])
                v = eng.snap(r, min_val=0, max_val=SEQ_PER_CORE * MAX_SL - RU)
                eng.dma_start(
                    out=y[bass.ds(v, RU), :],
                    in_=x[n_base + m * RU : n_base + (m + 1) * RU, :],
                ).then_inc(sem_rem, 16)

        half = n_rem // 2

        @block.scalar
        def _(scalar):
            # tiny table load first (unblocks the remainder copies); its
            # completion stall is hidden behind the already-issued big copies
            scalar.dma_start(out=dst_t[:, :], in_=dst[:, :]).then_inc(sem_tab, 16)
            big(scalar, 1, lead=LEAD)
            scalar.wait_ge(sem_tab, 16)
            with scalar.register("dst_row_act") as r:
                smalls(scalar, range(half, n_rem), r)
            big(scalar, 3)

        @block.sync
        def _(sync):
            big(sync, 0, lead=LEAD)
            sync.wait_ge(sem_tab, 16)
            with sync.register("dst_row_sp") as r:
                smalls(sync, range(half), r)
            big(sync, 2)
            sync.wait_ge(sem_big, 16 * (len(bases) + 2))
            sync.wait_ge(sem_rem, 16 * n_rem)
    return nc


def _groups():
    return [[k, 15 - k, 16 + k, 31 - k] for k in range(NCORES)]


def _host_fallback(S, L, max_sl):
    out = np.zeros((len(L), max_sl, S.shape[1]), dtype=S.dtype)
    off = 0
    for b, ln in enumerate(L):
        out[b, :ln] = S[off : off + ln]
        off += ln
    return out


def _fast_path_ok(S, L, max_sl):
    if (
        max_sl != MAX_SL
        or len(L) != B
        or S.shape[1] != F
        or int(L.sum()) != S.shape[0]
        or np.any(L % RU)
        or np.any(L < RU)
        or np.any(L > max_sl)
    ):
        return False
    groups = _groups()
    totals = [sum(int(L[s]) for s in g) for g in groups]
    if len(set(totals)) != 1:
        return False
    bases = [min(int(L[g[j]]) for g in groups) for j in range(SEQ_PER_CORE)]
    rem = totals[0] - sum(bases)
    if rem % RU or not (1 <= rem // RU <= 64):
        return False
    if min(bases) < 128:  # lead-off split in _build_nc needs bases >= LEAD
        return False
    return True


def _prepare(S, L):
    offsets = np.zeros(B + 1, dtype=np.int64)
    np.cumsum(L, out=offsets[1:])
    groups = _groups()
    bases = [min(int(L[g[j]]) for g in groups) for j in range(SEQ_PER_CORE)]
    rem_rows = sum(int(L[s]) for s in groups[0]) - sum(bases)
    n_rem = rem_rows // RU

    in_maps = []
    for k in range(NCORES):
        xs = []
        tails = []
        dst_k = np.zeros((1, n_rem), dtype=np.int32)
        p = 0
        for j, s in enumerate(groups[k]):
            ln = int(L[s])
            bj = bases[j]
            xs.append(S[offsets[s] : offsets[s] + bj])
            tails.append(S[offsets[s] + bj : offsets[s] + ln])
            for u in range((ln - bj) // RU):
                dst_k[0, p] = j * MAX_SL + bj + u * RU
                p += 1
        assert p == n_rem
        x_k = np.concatenate(xs + tails, axis=0)
        in_maps.append({"x": x_k, "dst": dst_k})

    key = (tuple(bases), rem_rows)
    if key not in _NC_CACHE:
        _NC_CACHE[key] = _build_nc(*key)
    return _NC_CACHE[key], in_maps, groups


def _assemble(results, groups):
    out = np.empty((B, MAX_SL, F), dtype=np.float32)
    for k in range(NCORES):
        yk = np.asarray(results[k]["y"]).reshape(SEQ_PER_CORE, MAX_SL, F)
        for j, s in enumerate(groups[k]):
            out[s] = yk[j]
    return out


def kernel(concatenated_sequences, sequence_lengths, max_sl):
    S = np.ascontiguousarray(np.asarray(concatenated_sequences, dtype=np.float32))
    L = np.asarray(sequence_lengths).reshape(-1).astype(np.int64)
    max_sl = int(np.asarray(max_sl))

    if not _fast_path_ok(S, L, max_sl):
        return _host_fallback(S, L, max_sl)

    nc, in_maps, groups = _prepare(S, L)
    res = run_bass_kernel_spmd(nc, in_maps, list(range(NCORES))).results
    return _assemble(res, groups)


# revision 13
# speedup vs baseline: 1.1781x; 1.0030x over previous
"""Ragged -> padded batch scatter (BatchedSequences) on 8 TRN2 NeuronCores.

Reference semantics: rows of concatenated_sequences [T, F] are scattered into
a zero-padded output [B, max_sl, F] according to per-sequence lengths.

Strategy (pure data movement, memory-bound; per-NC HBM r+w sustains
~665 GB/s, so the floor for 2*25.4 MB per core is ~76.5 us of data phase
plus ~9.5 us framework ramp and ~3 us drain; measured 88.5-91 us per core
on quiet NCs, up to ~107 on NCs with neighbor HBM load. The previous
SBUF-staged indirect-scatter kernel measured 156 us: every byte crossed
the SDMA engines twice, capping at 16 engines x 26 GB/s):
  - Shard 4 sequences per core with the slot assignment
    groups[k] = [k, 15-k, 16+k, 31-k]; every core then owns exactly
    T/8 = 12416 rows (lengths decay linearly, slots pair them off).
  - Slot j's length varies per core, but its minimum over cores is a
    static "base" size (3648/3136/2624/2112 rows = 11520 of 12416 rows).
    Base pieces are copied with 4 big *direct DRAM->DRAM* DMAs whose
    sizes/offsets are identical on all 8 cores (pure SPMD), split over
    the two HWDGE rings (sync + scalar). No SBUF transit: each byte
    crosses the SDMA engines once instead of twice.
  - The ragged remainder (896 rows/core = 14 pieces of 64 rows) is also
    DRAM->DRAM: gpsimd loads each piece's destination row from a tiny
    per-core int32 table (SBUF) into a register and issues a
    dynamic-offset SWDGE copy. These interleave with the big copies on
    the SDMA engines mid-stream.
  - Host pre-arranges each core's x so all source offsets are static:
    [base_0 | base_1 | base_2 | base_3 | tail_0 | tail_1 | tail_2 | tail_3].
  - Padding stays zero because run_bass_kernel_spmd pre-zeroes / donates
    zero-filled ExternalOutput buffers.
"""

from contextlib import ExitStack

import numpy as np

import concourse.bass as bass
import concourse.mybir as mybir
from concourse.bass_utils import run_bass_kernel_spmd

B = 32
F = 512
MAX_SL = 4096
NCORES = 8
SEQ_PER_CORE = B // NCORES
RU = 64                         # remainder piece: 64 rows = 128 KiB

_NC_CACHE: dict[tuple, bass.Bass] = {}


def _build_nc(bases: tuple[int, ...], rem_rows: int) -> bass.Bass:
    """Uniform per-core program (see module docstring)."""
    nc = bass.Bass()
    n_base = sum(bases)
    n_rows = n_base + rem_rows
    n_rem = rem_rows // RU
    x = nc.declare_dram_parameter("x", [n_rows, F], mybir.dt.float32, isOutput=False)
    dst = nc.declare_dram_parameter("dst", [1, n_rem], mybir.dt.int32, isOutput=False)
    y = nc.declare_dram_parameter(
        "y", [SEQ_PER_CORE * MAX_SL, F], mybir.dt.float32, isOutput=True
    )

    src_off = [0]
    for bj in bases:
        src_off.append(src_off[-1] + bj)

    with ExitStack() as ctx:
        dst_t = ctx.enter_context(nc.sbuf_tensor([1, n_rem], mybir.dt.int32))
        sem_tab = ctx.enter_context(nc.semaphore("sem_tab"))
        sem_data = ctx.enter_context(nc.semaphore("sem_data"))
        block = ctx.enter_context(nc.Block())

        LEAD = 128  # small lead-off piece: fast descriptor gen -> engines start sooner

        def big(eng, j, lead=0):
            if lead:
                eng.dma_start(
                    out=y[j * MAX_SL : j * MAX_SL + lead, :],
                    in_=x[src_off[j] : src_off[j] + lead, :],
                ).then_inc(sem_data, 16)
            eng.dma_start(
                out=y[j * MAX_SL + lead : j * MAX_SL + bases[j], :],
                in_=x[src_off[j] + lead : src_off[j] + bases[j], :],
            ).then_inc(sem_data, 16)

        def smalls(eng, ms, r):
            # dynamic-offset DRAM->DRAM remainder copies on the HWDGE ring,
            # placed mid-ring so they drain between the big copies
            for m in ms:
                eng.reg_load(r, dst_t[0:1, m : m + 1])
                v = eng.snap(r, min_val=0, max_val=SEQ_PER_CORE * MAX_SL - RU)
                eng.dma_start(
                    out=y[bass.ds(v, RU), :],
                    in_=x[n_base + m * RU : n_base + (m + 1) * RU, :],
                ).then_inc(sem_data, 16)

        half = n_rem // 2

        @block.scalar
        def _(scalar):
            # tiny table load first (unblocks the remainder copies); its
            # completion stall is hidden behind the already-issued big copies
            scalar.dma_start(out=dst_t[:, :], in_=dst[:, :]).then_inc(sem_tab, 16)
            big(scalar, 1, lead=LEAD)
            scalar.wait_ge(sem_tab, 16)
            with scalar.register("dst_row_act") as r:
                smalls(scalar, range(half, n_rem), r)
            big(scalar, 3)

        @block.sync
        def _(sync):
            big(sync, 0, lead=LEAD)
            sync.wait_ge(sem_tab, 16)
            with sync.register("dst_row_sp") as r:
                smalls(sync, range(half), r)
            big(sync, 2)
            sync.wait_ge(sem_data, 16 * (len(bases) + 2 + n_rem))
    return nc


def _groups():
    return [[k, 15 - k, 16 + k, 31 - k] for k in range(NCORES)]


def _host_fallback(S, L, max_sl):
    out = np.zeros((len(L), max_sl, S.shape[1]), dtype=S.dtype)
    off = 0
    for b, ln in enumerate(L):
        out[b, :ln] = S[off : off + ln]
        off += ln
    return out


def _fast_path_ok(S, L, max_sl):
    if (
        max_sl != MAX_SL
        or len(L) != B
        or S.shape[1] != F
        or int(L.sum()) != S.shape[0]
        or np.any(L % RU)
        or np.any(L < RU)
        or np.any(L > max_sl)
    ):
        return False
    groups = _groups()
    totals = [sum(int(L[s]) for s in g) for g in groups]
    if len(set(totals)) != 1:
        return False
    bases = [min(int(L[g[j]]) for g in groups) for j in range(SEQ_PER_CORE)]
    rem = totals[0] - sum(bases)
    if rem % RU or not (1 <= rem // RU <= 64):
        return False
    if min(bases) < 128:  # lead-off split in _build_nc needs bases >= LEAD
        return False
    return True


def _prepare(S, L):
    offsets = np.zeros(B + 1, dtype=np.int64)
    np.cumsum(L, out=offsets[1:])
    groups = _groups()
    bases = [min(int(L[g[j]]) for g in groups) for j in range(SEQ_PER_CORE)]
    rem_rows = sum(int(L[s]) for s in groups[0]) - sum(bases)
    n_rem = rem_rows // RU

    in_maps = []
    for k in range(NCORES):
        xs = []
        tails = []
        dst_k = np.zeros((1, n_rem), dtype=np.int32)
        p = 0
        for j, s in enumerate(groups[k]):
            ln = int(L[s])
            bj = bases[j]
            xs.append(S[offsets[s] : offsets[s] + bj])
            tails.append(S[offsets[s] + bj : offsets[s] + ln])
            for u in range((ln - bj) // RU):
                dst_k[0, p] = j * MAX_SL + bj + u * RU
                p += 1
        assert p == n_rem
        x_k = np.concatenate(xs + tails, axis=0)
        in_maps.append({"x": x_k, "dst": dst_k})

    key = (tuple(bases), rem_rows)
    if key not in _NC_CACHE:
        _NC_CACHE[key] = _build_nc(*key)
    return _NC_CACHE[key], in_maps, groups


def _assemble(results, groups):
    out = np.empty((B, MAX_SL, F), dtype=np.float32)
    for k in range(NCORES):
        yk = np.asarray(results[k]["y"]).reshape(SEQ_PER_CORE, MAX_SL, F)
        for j, s in enumerate(groups[k]):
            out[s] = yk[j]
    return out


def kernel(concatenated_sequences, sequence_lengths, max_sl):
    S = np.ascontiguousarray(np.asarray(concatenated_sequences, dtype=np.float32))
    L = np.asarray(sequence_lengths).reshape(-1).astype(np.int64)
    max_sl = int(np.asarray(max_sl))

    if not _fast_path_ok(S, L, max_sl):
        return _host_fallback(S, L, max_sl)

    nc, in_maps, groups = _prepare(S, L)
    res = run_bass_kernel_spmd(nc, in_maps, list(range(NCORES))).results
    return _assemble(res, groups)
